# revision 45
# baseline (speedup 1.0000x reference)
"""Trainium2 Bass kernel for APPNP (k=2) + per-time-group standardization + MLP.

Strategy (8 NeuronCores, SPMD):
  - Nodes sharded by destination block: core c owns nodes [c*NL, (c+1)*NL).
  - Edges sharded by dst block; per hop each core gathers h_scaled[src] rows
    from a replicated (AllGather'd) HBM tensor via indirect DMA, and
    scatter-adds them into its 128-dst-node windows with one-hot
    selection-matrix matmuls accumulating in PSUM.
  - JJ norm: per-time-group stats via one-hot matmuls + AllReduce [G, 2D],
    applied via one-hot matmul broadcast-back.
  - 2-layer MLP + sigmoid on transposed tiles; output [OUT, NLP] per core.

kernel(**inputs) takes FULL inputs, shards internally, returns FULL output.
"""

import sys

sys.path.insert(0, "/opt/trn_rl_repo")

import numpy as np

from concourse import bass, bacc, mybir
import concourse.tile as tile
from concourse.masks import make_identity
from concourse import bass_utils

F32 = mybir.dt.float32
BF16 = mybir.dt.bfloat16
I32 = mybir.dt.int32
ALU = mybir.AluOpType
ACTF = mybir.ActivationFunctionType

try:
    import ml_dtypes
    NP_BF16 = ml_dtypes.bfloat16
except ImportError:  # pragma: no cover
    NP_BF16 = None


class Cfg:
    def __init__(self, N=50000, NC=8, D=128, G=40, OUT=40, ALPHA=0.5, EPS=1e-5, KH=2):
        assert N % NC == 0
        self.N, self.NC, self.D, self.G, self.OUT = N, NC, D, G, OUT
        self.ALPHA, self.EPS, self.KH = ALPHA, EPS, KH
        self.NL = N // NC                      # nodes per core (un-padded)
        self.NW = (self.NL + 127) // 128       # 128-node windows per core
        self.NLP = self.NW * 128               # padded nodes per core
        self.NGROWS = NC * 128 * self.NW       # rows of hs_full


FULL = Cfg()


# --------------------------------------------------------------------------
# Host-side preprocessing: integer index manipulation only (graph partition,
# CSR slotting, one-hot encodings, degree/group counts).
# --------------------------------------------------------------------------

def preprocess(cfg, features, w1, b1, w2, b2, src, dst, times):
    N, NC, NL, NW, NLP, G, D = (
        cfg.N, cfg.NC, cfg.NL, cfg.NW, cfg.NLP, cfg.G, cfg.D)

    src = np.asarray(src).astype(np.int64)
    dst = np.asarray(dst).astype(np.int64)
    times = np.asarray(times).astype(np.int64)
    features = np.asarray(features, dtype=np.float32)
    w1 = np.asarray(w1, dtype=np.float32)
    b1 = np.asarray(b1, dtype=np.float32)
    w2 = np.asarray(w2, dtype=np.float32)
    b2 = np.asarray(b2, dtype=np.float32)

    deg_out = np.bincount(src, minlength=N).astype(np.float32)
    deg_in = np.bincount(dst, minlength=N).astype(np.float32)
    cnt = np.bincount(times, minlength=G).astype(np.float32)[:G]

    # hs_full row index of a source node (window-major padded layout)
    c_s, r_s = src // NL, src % NL
    g_row = (c_s * 128 + (r_s % 128)) * NW + (r_s // 128)

    edge_core = dst // NL

    # The gather table is split into two window-aligned halves so
    # dma_gather's int16 indices reach every row AND so each half can be
    # AllGather'd separately (pipelined with compute):
    #   A = windows [0, WA) of every core, B = windows [WA, NW).
    WA = (NW + 1) // 2            # windows in half A
    NA = WA * 128                 # rows per core in half A
    NB = NLP - NA

    # First pass: per-(core, window, half) edge counts -> global T2
    per_core = []
    T2 = 1
    for c in range(NC):
        m = edge_core == c
        ed = dst[m]
        sr = src[m]
        c_s2, r_s2 = sr // NL, sr % NL
        dloc = ed - c * NL
        w = dloc // 128
        dw = dloc % 128
        half = (r_s2 >= NA).astype(np.int64)
        gr = np.where(half == 0, c_s2 * NA + r_s2, c_s2 * NB + (r_s2 - NA))
        key = w * 2 + half
        order = np.argsort(key, kind="stable")
        key, w, dw, gr, half = (key[order], w[order], dw[order], gr[order],
                                half[order])
        counts = np.bincount(key, minlength=NW * 2)
        T2 = max(T2, int(np.ceil(counts.max() / 128)))
        per_core.append((key, w, dw, gr, half, counts))
    T = 2 * T2  # matmul tiles per window
    def to_part_major(arr_nodes_x):
        # [NLP, X] node-major -> [128, NW*X] partition-major window layout
        X = arr_nodes_x.shape[1]
        return (arr_nodes_x.reshape(NW, 128, X)
                .transpose(1, 0, 2).reshape(128, NW * X))

    def wrap_idx(flat_nw):  # [NW, T2*128] -> [128, NW*T2*8] int16
        F = flat_nw.reshape(NW, T2 * 8, 16)
        return np.tile(F.transpose(2, 0, 1).reshape(16, NW * T2 * 8),
                       (8, 1)).astype(np.int16)

    in_maps = []
    for c in range(NC):
        key, w, dw, gr, half, counts = per_core[c]
        starts = np.zeros(NW * 2, np.int64)
        starts[1:] = np.cumsum(counts)[:-1]
        sl = np.arange(len(w)) - starts[key]   # slot within (window, half)
        p = sl % 128
        t = sl // 128
        # index tensors, one per half, window-blocked then 16-wrapped.
        # Valid slots first, then trailing -1 (skipped by the Q7 desc-gen);
        # the per-gather valid count (rounded up to 128 so every SDMA engine
        # still gets descriptors for its completion sem) is shipped in gcnt.
        cnt_lo = counts[0::2]
        cnt_hi = counts[1::2]
        rnd_lo = np.minimum(np.ceil(np.maximum(cnt_lo, 1) / 128) * 128,
                            T2 * 128).astype(np.int64)
        rnd_hi = np.minimum(np.ceil(np.maximum(cnt_hi, 1) / 128) * 128,
                            T2 * 128).astype(np.int64)
        flat_lo = np.full((NW, T2 * 128), -1, np.int64)
        flat_hi = np.full((NW, T2 * 128), -1, np.int64)
        for wi in range(NW):
            flat_lo[wi, cnt_lo[wi]:rnd_lo[wi]] = 0
            flat_hi[wi, cnt_hi[wi]:rnd_hi[wi]] = 0
        is_lo = half == 0
        flat_lo[w[is_lo], sl[is_lo]] = gr[is_lo]
        flat_hi[w[~is_lo], sl[~is_lo]] = gr[~is_lo]
        gl = wrap_idx(flat_lo)
        gh = wrap_idx(flat_hi)
        gcnt = np.zeros((1, NW * 2), np.int32)
        gcnt[0, 0::2] = rnd_lo
        gcnt[0, 1::2] = rnd_hi
        # dst-in-window per matmul tile: window tiles = [lo 0..T2) [hi T2..2T2)
        col = w * T + half * T2 + t
        dstw = np.full((128, NW * T), -1.0, np.float32)
        dstw[p, col] = dw.astype(np.float32)
        dstw = dstw.astype(NP_BF16)  # values in [-1, 127]: exact in bf16

        lo, hi = c * NL, (c + 1) * NL
        feat_pad = np.zeros((NLP, D), np.float32)
        feat_pad[:NL] = features[lo:hi]
        dego_pad = np.zeros((NLP, 1), np.float32)
        dego_pad[:NL, 0] = deg_out[lo:hi]
        degi_pad = np.zeros((NLP, 1), np.float32)
        degi_pad[:NL, 0] = deg_in[lo:hi]
        t_loc = times[lo:hi]
        tw = np.zeros((NLP, G), np.float32)
        tw[np.arange(NL), t_loc] = 1.0
        in_maps.append({
            "feat": to_part_major(feat_pad),
            "degout": to_part_major(dego_pad),
            "degin": to_part_major(degi_pad),
            "ga": gl,
            "gb": gh,
            "gcnt": gcnt,
            "dstw": dstw,
            "tw": to_part_major(tw),
            "twT": np.ascontiguousarray(tw.T).astype(NP_BF16),
            "cnt": cnt.reshape(G, 1),
            "w1": w1,
            "w2": w2,
            "b1": b1.reshape(-1, 1),
            "b2": b2.reshape(-1, 1),
        })
    return in_maps, T


# --------------------------------------------------------------------------
# Bass program (identical on all 8 cores; data differs via in_maps)
# --------------------------------------------------------------------------

def _rsqrt_refined(nc, wp, out, x, shape, tag):
    """out = 1/sqrt(x), ACT-LUT seeded + one Newton step (y*(1.5-0.5*x*y^2)).

    The ACT Sqrt LUT has a loose precision budget (~65536 ULP) and the DVE
    reciprocal is also approximate; one Newton step on rsqrt fixes both."""
    s = wp.tile(shape, F32, tag=tag + "_s")
    nc.scalar.activation(s[:], x, ACTF.Sqrt)
    y0 = wp.tile(shape, F32, tag=tag + "_y")
    nc.vector.reciprocal(y0[:], s[:])
    t = wp.tile(shape, F32, tag=tag + "_t")
    nc.vector.tensor_tensor(t[:], y0[:], y0[:], op=ALU.mult)
    nc.vector.tensor_tensor(t[:], t[:], x, op=ALU.mult)
    # t = 1.5 - 0.5*t  (fused two-op tensor_scalar)
    nc.vector.tensor_scalar(t[:], t[:], -0.5, 1.5, ALU.mult, ALU.add)
    nc.vector.tensor_tensor(out, y0[:], t[:], op=ALU.mult)


def _recip_refined(nc, wp, out, x, shape, tag):
    """out = 1/x with one Newton step: y*(2 - x*y)."""
    y0 = wp.tile(shape, F32, tag=tag + "_y")
    nc.vector.reciprocal(y0[:], x)
    t = wp.tile(shape, F32, tag=tag + "_t")
    nc.vector.tensor_tensor(t[:], y0[:], x, op=ALU.mult)
    nc.vector.tensor_scalar(t[:], t[:], -1.0, 2.0, ALU.mult, ALU.add)
    nc.vector.tensor_tensor(out, y0[:], t[:], op=ALU.mult)


def build_nc(cfg, T, debug=False):
    N, NC, NL, NW, NLP, G, D, OUT = (
        cfg.N, cfg.NC, cfg.NL, cfg.NW, cfg.NLP, cfg.G, cfg.D, cfg.OUT)
    ALPHA, EPS = cfg.ALPHA, cfg.EPS
    RG = [list(range(NC))]
    T2 = T // 2
    WA = (NW + 1) // 2
    NA = WA * 128
    NB = NLP - NA
    I16 = mybir.dt.int16

    nc = bacc.Bacc("TRN2", target_bir_lowering=False, debug=False,
                   num_devices=NC, num_swdge_queues=4)

    feat_d = nc.dram_tensor("feat", [128, NLP], F32, kind="ExternalInput")
    dego_d = nc.dram_tensor("degout", [128, NW], F32, kind="ExternalInput")
    degi_d = nc.dram_tensor("degin", [128, NW], F32, kind="ExternalInput")
    ga_d = nc.dram_tensor("ga", [128, NW * T2 * 8], I16, kind="ExternalInput")
    gb_d = nc.dram_tensor("gb", [128, NW * T2 * 8], I16, kind="ExternalInput")
    dstw_d = nc.dram_tensor("dstw", [128, NW * T], BF16, kind="ExternalInput")
    tw_d = nc.dram_tensor("tw", [128, NW * G], F32, kind="ExternalInput")
    twT_d = nc.dram_tensor("twT", [G, NLP], BF16, kind="ExternalInput")
    gcnt_d = nc.dram_tensor("gcnt", [1, NW * 2], I32, kind="ExternalInput")
    cnt_d = nc.dram_tensor("cnt", [G, 1], F32, kind="ExternalInput")
    w1_d = nc.dram_tensor("w1", [D, D], F32, kind="ExternalInput")
    w2_d = nc.dram_tensor("w2", [OUT, D], F32, kind="ExternalInput")
    b1_d = nc.dram_tensor("b1", [D, 1], F32, kind="ExternalInput")
    b2_d = nc.dram_tensor("b2", [OUT, 1], F32, kind="ExternalInput")
    out_d = nc.dram_tensor("out", [OUT, NLP], F32, kind="ExternalOutput")
    if debug:
        dbg_hs0 = nc.dram_tensor("dbg_hs0", [128, NLP], F32,
                                 kind="ExternalOutput")
        dbg_hs1 = nc.dram_tensor("dbg_hs1", [128, NLP], F32,
                                 kind="ExternalOutput")
        dbg_h2 = nc.dram_tensor("dbg_h2", [128, NLP], F32,
                                 kind="ExternalOutput")
        dbg_st = nc.dram_tensor("dbg_st", [G, 2 * D], F32,
                                 kind="ExternalOutput")
        dbg_sel = nc.dram_tensor("dbg_sel", [G, 2 * D], F32,
                                 kind="ExternalOutput")

    # internal DRAM (A/B window-halves, per hop)
    bncA0 = nc.dram_tensor("bncA0", [NA, D], BF16)
    bncB0 = nc.dram_tensor("bncB0", [NB, D], BF16)
    bncA1 = nc.dram_tensor("bncA1", [NA, D], BF16)
    bncB1 = nc.dram_tensor("bncB1", [NB, D], BF16)
    hsfA0 = nc.dram_tensor("hsfA0", [NC * NA, D], BF16, addr_space="Shared")
    hsfB0 = nc.dram_tensor("hsfB0", [NC * NB, D], BF16, addr_space="Shared")
    hsfA1 = nc.dram_tensor("hsfA1", [NC * NA, D], BF16, addr_space="Shared")
    hsfB1 = nc.dram_tensor("hsfB1", [NC * NB, D], BF16, addr_space="Shared")
    stin = nc.dram_tensor("statin", [G, 2 * D], F32)
    stout = nc.dram_tensor("statout", [G, 2 * D], F32, addr_space="Shared")

    with tile.TileContext(nc) as tc:
        with tc.tile_pool(name="const", bufs=1) as cp, \
             tc.tile_pool(name="work", bufs=2) as wp, \
             tc.tile_pool(name="big", bufs=2) as bp, \
             tc.tile_pool(name="chunk", bufs=6) as chp:

            # ---------------- setup ----------------
            # Critical path to the first AllGather: feat/deg loads -> norms
            # -> hs0 -> bounce DMA. Everything else is created after so the
            # scheduler keeps it off that path.
            feat = cp.tile([128, NLP], F32, name="feat_sb")
            nc.sync.dma_start(out=feat[:], in_=feat_d[:, :])
            dego = cp.tile([128, NW], F32, name="dego_sb")
            nc.sync.dma_start(out=dego[:], in_=dego_d[:, :])
            degi = cp.tile([128, NW], F32, name="degi_sb")
            nc.sync.dma_start(out=degi[:], in_=degi_d[:, :])

            nout = cp.tile([128, NW], F32, name="nout")
            nin = cp.tile([128, NW], F32, name="nin")
            tmpd = wp.tile([128, NW], F32, tag="tmpd")
            nc.vector.tensor_scalar_max(tmpd[:], dego[:], 1.0)
            _rsqrt_refined(nc, wp, nout[:], tmpd[:], [128, NW], "no")
            tmpd2 = wp.tile([128, NW], F32, tag="tmpd")
            nc.vector.tensor_scalar_max(tmpd2[:], degi[:], 1.0)
            _rsqrt_refined(nc, wp, nin[:], tmpd2[:], [128, NW], "ni")
            nc.vector.tensor_scalar_mul(nin[:], nin[:], 1.0 - ALPHA)

            hs0 = bp.tile([128, NLP], BF16, tag="bigh")
            nc.vector.tensor_tensor(
                hs0[:].rearrange("p (w d) -> p w d", d=128),
                feat[:].rearrange("p (w d) -> p w d", d=128),
                nout[:].to_broadcast([128, NW, 128]),
                op=ALU.mult)
            bvA0 = bncA0.ap().rearrange("(w p) d -> p w d", p=128)
            bvB0 = bncB0.ap().rearrange("(w p) d -> p w d", p=128)
            nc.sync.dma_start(
                out=bvA0, in_=hs0[:, 0:NA].rearrange("p (w d) -> p w d", d=128))
            nc.sync.dma_start(
                out=bvB0,
                in_=hs0[:, NA:NLP].rearrange("p (w d) -> p w d", d=128))
            nc.gpsimd.collective_compute(
                "AllGather", ALU.bypass, replica_groups=RG,
                ins=[bncA0.ap().opt()], outs=[hsfA0.ap().opt()])
            nc.gpsimd.collective_compute(
                "AllGather", ALU.bypass, replica_groups=RG,
                ins=[bncB0.ap().opt()], outs=[hsfB0.ap().opt()])
            if debug:
                nc.sync.dma_start(out=dbg_hs0[:, :], in_=hs0[:])

            # warmup gather: pays the one-time Q7 ext-isa IRAM load early
            wsrc = feat_d.ap().rearrange("p (w d) -> (p w) d", d=128)
            wu_i = cp.tile([128, 8], mybir.dt.int16, name="wu_i")
            nc.vector.memset(wu_i[:], 0)
            wu_o = cp.tile([128, 128], F32, name="wu_o")
            nc.gpsimd.dma_gather(
                out_ap=wu_o[:].rearrange("p (t d) -> p t d", d=128),
                in_ap=wsrc, idxs_ap=wu_i[:],
                num_idxs=128, num_idxs_reg=128, elem_size=128)

            # --- non-critical setup ---
            ga = cp.tile([128, NW * T2 * 8], I16, name="ga_sb")
            nc.sync.dma_start(out=ga[:], in_=ga_d[:, :])
            gb = cp.tile([128, NW * T2 * 8], I16, name="gb_sb")
            nc.sync.dma_start(out=gb[:], in_=gb_d[:, :])
            dstw = cp.tile([128, NW * T], BF16, name="dstw_sb")
            nc.sync.dma_start(out=dstw[:], in_=dstw_d[:, :])
            tw = cp.tile([128, NW * G], F32, name="tw_sb")
            nc.sync.dma_start(out=tw[:], in_=tw_d[:, :])
            twT = cp.tile([G, NLP], BF16, name="twT_sb")
            nc.sync.dma_start(out=twT[:], in_=twT_d[:, :])
            gcnt = cp.tile([1, NW * 2], I32, name="gcnt_sb")
            nc.sync.dma_start(out=gcnt[:], in_=gcnt_d[:, :])
            cntregs = [nc.gpsimd.alloc_register(f"gcnt_r{i}")
                       for i in range(8)]
            w1s = cp.tile([D, D], F32, name="w1_sb")
            nc.sync.dma_start(out=w1s[:], in_=w1_d[:, :])
            w2s = cp.tile([OUT, D], F32, name="w2_sb")
            nc.sync.dma_start(out=w2s[:], in_=w2_d[:, :])
            b1s = cp.tile([D, 1], F32, name="b1_sb")
            nc.sync.dma_start(out=b1s[:], in_=b1_d[:, :])
            b2s = cp.tile([OUT, 1], F32, name="b2_sb")
            nc.sync.dma_start(out=b2s[:], in_=b2_d[:, :])
            cnts = cp.tile([G, 1], F32, name="cnt_sb")
            nc.sync.dma_start(out=cnts[:], in_=cnt_d[:, :])

            # pre-zero the gather-chunk slots: tiles past a window's valid
            # count are never DMA-written, and PE 0*garbage must stay finite
            for _i in range(8):
                _m = chp.tile([128, T2 * 128], BF16, tag="mA", bufs=8)
                nc.gpsimd.memset(_m[:], 0.0)
            for _i in range(3):
                _m = chp.tile([128, T2 * 128], BF16, tag="mB", bufs=3)
                nc.gpsimd.memset(_m[:], 0.0)

            iota_i = cp.tile([128, T * 128], I32, name="iota_i")
            nc.gpsimd.iota(iota_i[:], pattern=[[0, T], [1, 128]], base=0,
                           channel_multiplier=0)
            iota_b = cp.tile([128, T * 128], BF16, name="iota_b")
            nc.vector.tensor_copy(iota_b[:], iota_i[:])

            ident = cp.tile([128, 128], F32, name="ident")
            make_identity(nc, ident[:])

            # transpose MLP weights once
            with tc.tile_pool(name="pset", bufs=1, space="PSUM") as pset:
                w1T = cp.tile([D, D], BF16, name="w1T")
                tp = pset.tile([D, D], F32, tag="tp")
                nc.tensor.transpose(tp[:], w1s[:], ident[:])
                nc.vector.tensor_copy(w1T[:], tp[:])
                w2T = cp.tile([D, OUT], BF16, name="w2T")
                tp2 = pset.tile([D, OUT], F32, tag="tp")
                nc.tensor.transpose(tp2[:], w2s[:], ident[:OUT, :OUT])
                nc.vector.tensor_copy(w2T[:], tp2[:])
                identb = cp.tile([128, 128], BF16, name="identb")
                nc.vector.tensor_copy(identb[:], ident[:])

            # featA = alpha * h0 (in place; raw feat no longer needed)
            nc.vector.tensor_scalar_mul(feat[:], feat[:], ALPHA)

            hs1 = bp.tile([128, NLP], BF16, tag="bigh")


            # ---------------- propagation hops ----------------
            PRE = 6  # windows of A-gather prefetch (hides AG_B latency)
            gqc = [1]  # global gather counter (warmup was 0); queue=cnt%4

            def issue(src, gidx_sb, w, mpool, mtag, bufs):
                m = mpool.tile([128, T2 * 128], BF16, tag=mtag, bufs=bufs)
                r = cntregs[gqc[0] % 8]
                k = 2 * w if mtag == "mA" else 2 * w + 1
                nc.gpsimd.reg_load(r, gcnt[0:1, k:k + 1])
                nc.gpsimd.dma_gather(
                    out_ap=m[:].rearrange("p (t d) -> p t d", d=128),
                    in_ap=src[:, :],
                    idxs_ap=gidx_sb[:, w * T2 * 8:(w + 1) * T2 * 8],
                    num_idxs=T2 * 128, num_idxs_reg=r,
                    elem_size=128, queue_num=gqc[0] % 4)
                gqc[0] += 1
                return m

            def hop(srcA, srcB, is_last, h2=None, stats=None, ag_next=None):
                with tc.tile_pool(name="pagg", bufs=2, space="PSUM") as pagg:
                    pend = {}
                    for w in range(min(PRE, NW)):
                        pend[w] = issue(srcA, ga, w, chp, "mA", 8)
                    for w in range(NW):
                        mB = issue(srcB, gb, w, chp, "mB", 3)
                        if w + PRE < NW:
                            pend[w + PRE] = issue(srcA, ga, w + PRE,
                                                  chp, "mA", 8)
                        mA = pend.pop(w)
                        agg = pagg.tile([128, 128], F32, tag="agg")
                        st = wp.tile([128, T * 128], BF16, tag="st", bufs=3)
                        nc.vector.tensor_tensor(
                            st[:].rearrange("p (t d) -> p t d", d=128),
                            iota_b[:].rearrange("p (t d) -> p t d", d=128),
                            dstw[:, w * T:(w + 1) * T].to_broadcast(
                                [128, T, 128]),
                            op=ALU.is_equal)
                        for t in range(T):
                            rhs = (mA[:, t * 128:(t + 1) * 128] if t < T2 else
                                   mB[:, (t - T2) * 128:(t - T2 + 1) * 128])
                            nc.tensor.matmul(
                                agg[:], lhsT=st[:, t * 128:(t + 1) * 128],
                                rhs=rhs, start=(t == 0), stop=(t == T - 1))
                        s = slice(w * 128, (w + 1) * 128)
                        hw = wp.tile([128, 128], F32, tag="hw", bufs=3)
                        nc.vector.tensor_scalar(hw[:], agg[:],
                                                nin[:, w:w + 1], None, ALU.mult)
                        if is_last:
                            nc.vector.tensor_tensor(h2[:, s], hw[:], feat[:, s],
                                                    op=ALU.add)
                            stp1, stp2 = stats
                            sq = wp.tile([128, 128], F32, tag="sq", bufs=2)
                            nc.scalar.activation(sq[:], h2[:, s], ACTF.Square)
                            lt = tw[:, w * G:(w + 1) * G]
                            nc.tensor.matmul(stp1[:], lhsT=lt, rhs=h2[:, s],
                                             start=(w == 0), stop=(w == NW - 1))
                            nc.tensor.matmul(stp2[:], lhsT=lt, rhs=sq[:],
                                             start=(w == 0), stop=(w == NW - 1))
                        else:
                            h1t = wp.tile([128, 128], F32, tag="hw", bufs=3)
                            nc.vector.tensor_tensor(h1t[:], hw[:], feat[:, s],
                                                    op=ALU.add)
                            nc.vector.tensor_scalar(hs1[:, s], h1t[:],
                                                    nout[:, w:w + 1], None,
                                                    ALU.mult)
                            if w < WA:
                                nc.sync.dma_start(
                                    out=bncA1.ap()[w * 128:(w + 1) * 128, :],
                                    in_=hs1[:, s])
                            else:
                                wb = w - WA
                                nc.sync.dma_start(
                                    out=bncB1.ap()[wb * 128:(wb + 1) * 128, :],
                                    in_=hs1[:, s])
                            if w == WA - 1:
                                # first half of hs1 is complete: overlap its
                                # AllGather with the rest of this hop
                                nc.gpsimd.collective_compute(
                                    "AllGather", ALU.bypass,
                                    replica_groups=RG,
                                    ins=[bncA1.ap().opt()],
                                    outs=[hsfA1.ap().opt()])

            hop(hsfA0, hsfB0, is_last=False)
            if debug:
                nc.sync.dma_start(out=dbg_hs1[:, :], in_=hs1[:])
            nc.gpsimd.collective_compute(
                "AllGather", ALU.bypass, replica_groups=RG,
                ins=[bncB1.ap().opt()], outs=[hsfB1.ap().opt()])
            h2 = bp.tile([128, NLP], F32, tag="big")
            with tc.tile_pool(name="pstat", bufs=1, space="PSUM") as pstat:
                stp1 = pstat.tile([G, D], F32, tag="stats1")
                stp2 = pstat.tile([G, D], F32, tag="stats2")
                hop(hsfA1, hsfB1, is_last=True, h2=h2, stats=(stp1, stp2))
                stsb = cp.tile([G, 2 * D], F32, name="stsb")
                nc.vector.tensor_copy(stsb[:, 0:D], stp1[:])
                nc.vector.tensor_copy(stsb[:, D:2 * D], stp2[:])
            if debug:
                nc.sync.dma_start(out=dbg_h2[:, :], in_=h2[:])
            nc.sync.dma_start(out=stin[:, :], in_=stsb[:])
            nc.gpsimd.collective_compute(
                "AllReduce", ALU.add, replica_groups=RG,
                ins=[stin.ap().opt()], outs=[stout.ap().opt()])
            stg = cp.tile([G, 2 * D], F32, name="stg")
            nc.sync.dma_start(out=stg[:], in_=stout[:, :])
            if debug:
                nc.sync.dma_start(out=dbg_st[:, :], in_=stg[:])

            # moments -> selrhs = [mean | rstd]  [G, 2D]
            selrhs = cp.tile([G, 2 * D], BF16, name="selrhs")
            cc = wp.tile([G, 1], F32, tag="cc")
            nc.vector.tensor_scalar_max(cc[:], cnts[:], 1.0)
            rc = wp.tile([G, 1], F32, tag="rc")
            _recip_refined(nc, wp, rc[:], cc[:], [G, 1], "rc")
            nc.vector.tensor_scalar(selrhs[:, 0:D], stg[:, 0:D], rc[:],
                                    None, ALU.mult)
            ex2 = wp.tile([G, D], F32, tag="ex2")
            nc.vector.tensor_scalar(ex2[:], stg[:, D:2 * D], rc[:],
                                    None, ALU.mult)
            m2 = wp.tile([G, D], F32, tag="m2")
            nc.vector.tensor_tensor(m2[:], selrhs[:, 0:D], selrhs[:, 0:D],
                                    op=ALU.mult)
            nc.vector.tensor_tensor(ex2[:], ex2[:], m2[:], op=ALU.subtract)
            nc.vector.tensor_scalar(ex2[:], ex2[:], 0.0, EPS, ALU.max, ALU.add)
            _rsqrt_refined(nc, wp, selrhs[:, D:2 * D], ex2[:], [G, D], "rs")

            if debug:
                nc.sync.dma_start(out=dbg_sel[:, :], in_=selrhs[:])

            # ---------------- apply + MLP ----------------
            with tc.tile_pool(name="psel", bufs=2, space="PSUM") as psel, \
                 tc.tile_pool(name="ptr", bufs=2, space="PSUM") as ptr, \
                 tc.tile_pool(name="pz1", bufs=2, space="PSUM") as pz1, \
                 tc.tile_pool(name="pz2", bufs=2, space="PSUM") as pz2:
                for w in range(NW):
                    s = slice(w * 128, (w + 1) * 128)
                    sel = psel.tile([128, 2 * D], F32, tag="sel")
                    nc.tensor.matmul(sel[:], lhsT=twT[:, s], rhs=selrhs[:],
                                     start=True, stop=True)
                    h2nf = wp.tile([128, 128], F32, tag="h2n", bufs=3)
                    nc.vector.tensor_tensor(h2nf[:], h2[:, s], sel[:, 0:D],
                                            op=ALU.subtract)
                    h2n = wp.tile([128, 128], BF16, tag="h2nb", bufs=3)
                    nc.vector.tensor_tensor(h2n[:], h2nf[:], sel[:, D:2 * D],
                                            op=ALU.mult)
                    trp = ptr.tile([128, 128], BF16, tag="tr")
                    nc.tensor.transpose(trp[:], h2n[:], identb[:])
                    z0 = wp.tile([128, 128], BF16, tag="z0", bufs=3)
                    nc.scalar.activation(z0[:], trp[:], ACTF.Copy)
                    z1p = pz1.tile([D, 128], F32, tag="z1")
                    nc.tensor.matmul(z1p[:], lhsT=w1T[:], rhs=z0[:],
                                     start=True, stop=True)
                    z1 = wp.tile([D, 128], BF16, tag="z1s", bufs=3)
                    nc.scalar.activation(z1[:], z1p[:], ACTF.Relu, bias=b1s[:])
                    z2p = pz2.tile([OUT, 128], F32, tag="z2")
                    nc.tensor.matmul(z2p[:], lhsT=w2T[:], rhs=z1[:],
                                     start=True, stop=True)
                    o = wp.tile([OUT, 128], F32, tag="o", bufs=3)
                    nc.scalar.activation(o[:], z2p[:], ACTF.Sigmoid,
                                         bias=b2s[:])
                    nc.sync.dma_start(out=out_d[:, w * 128:(w + 1) * 128],
                                      in_=o[:])

    nc.compile()
    return nc


# --------------------------------------------------------------------------

_CACHE = {}


def _get_nc(cfg, T):
    key = (cfg.N, cfg.NC, cfg.G, cfg.OUT, T)
    if key not in _CACHE:
        _CACHE[key] = build_nc(cfg, T)
    return _CACHE[key]


def run(cfg, inputs, **run_kwargs):
    in_maps, T = preprocess(
        cfg, inputs["features"], inputs["w1"], inputs["b1"], inputs["w2"],
        inputs["b2"], inputs["src"], inputs["dst"], inputs["times"])
    nc = _get_nc(cfg, T)
    res = bass_utils.run_bass_kernel_spmd(
        nc, in_maps, core_ids=list(range(cfg.NC)), **run_kwargs)
    full = np.empty((cfg.N, cfg.OUT), np.float32)
    for c in range(cfg.NC):
        full[c * cfg.NL:(c + 1) * cfg.NL] = \
            res.results[c]["out"][:, :cfg.NL].T
    return full, res


def kernel(**inputs):
    out, _ = run(FULL, inputs)
    return out
